# revision 4
# baseline (speedup 1.0000x reference)
"""Trainium2 Bass kernel for nn_DynamicShortConvolution.

Reference computation (per token t, channel d):
    h    = silu(x @ w1)                       # [T, H]
    flat = h @ w2 + b2                        # [T, D*W]
    k    = flat.reshape(T, D, W)
    out[t, d] = silu(sum_w k[t, d, w] * x[t - (W-1) + w, d])

Sharding: 8 cores, each one (batch, half-of-T) shard of 2048 tokens plus a
3-token left halo.  Per-core tensors are TRANSPOSED ([D, T], channels on SBUF
partitions) so the causal shift is a free-dim offset and both matmuls run
without on-device transposes.

Schedule (v2):
  - DMA order w1, b2, x(16 tiles), w2(8 chunks) so mm1 starts ~4us in.
  - mm1 is dt-OUTER: each arriving x tile feeds one 8-matmul burst
    accumulating into 4 resident PSUM tiles; mm1 finishes right after the
    last x tile lands instead of serializing behind the full x load.
  - w2 is stored dt-major so mm2 group (dt,pi) only needs its own w2 chunk.
  - mm2 elementwise work is balanced across DVE / ACT / GPSIMD with a
    4-deep software pipeline so no engine queue ever stalls on same-group
    producers:
      DVE : stt tap0 (psum+bias)*x, products taps1-3, add m0+m1, final add
      ACT : evac taps1-3 (psum+bias -> sbuf bf16), final silu
      GPS : add m2+m3, odd-shift x copy (for 4B-aligned bf16 2x products)
"""

import numpy as np

# Problem constants (hardcoded per harness contract).
B, T, D, H, W = 4, 4096, 2048, 256, 4
HALO = W - 1
N_CORES = 8
TOK = (B * T) // N_CORES  # tokens per core = 2048


def _build_nc(tok, d, h, xstride):
    import concourse.bass as bass
    import concourse.bacc as bacc
    import concourse.mybir as mybir
    import concourse.tile as tile

    f32 = mybir.dt.float32
    bf16 = mybir.dt.bfloat16
    AF = mybir.ActivationFunctionType
    ALU = mybir.AluOpType

    n_dt = d // 128        # 16 d tiles
    n_hc = h // 128        # 2 h tiles
    P = 1024               # tokens per mm2 group
    n_pi = tok // P        # 2
    NG = n_dt * n_pi       # 32 groups

    nc = bacc.Bacc()

    # DRAM I/O (host-prepared layouts)
    xT = nc.declare_dram_parameter("xT", [n_dt, 128, xstride], bf16, isOutput=False)
    # w1d[p, dt*h + j] = w1[dt*128+p, j]
    w1d = nc.declare_dram_parameter("w1d", [128, n_dt * h], bf16, isOutput=False)
    # w2d[p, dt*1024 + hc*512 + w*128 + c] = w2[hc*128+p, (dt*128+c)*W + w]
    w2d = nc.declare_dram_parameter("w2d", [128, n_dt * 1024], bf16, isOutput=False)
    # b2d[p, dt*W + w] = b2[(dt*128+p)*W + w]
    b2d = nc.declare_dram_parameter("b2d", [128, n_dt * W], f32, isOutput=False)
    outT = nc.declare_dram_parameter("outT", [n_dt, 128, tok], bf16, isOutput=True)

    with tile.TileContext(nc) as tc:
        with (
            tc.tile_pool(name="resident", bufs=1) as rpool,
            tc.tile_pool(name="work", bufs=3) as wpool,
            tc.tile_pool(name="psum", bufs=4, space="PSUM") as ppool,
        ):
            # ---- resident tiles ----
            xT_sb = rpool.tile([128, n_dt * xstride], bf16, tag="xT")
            w1_sb = rpool.tile([128, n_dt * h], bf16, tag="w1")
            w2_sb = rpool.tile([128, n_dt * 1024], bf16, tag="w2")
            b2_sb = rpool.tile([128, n_dt * W], f32, tag="b2")
            hT_sb = rpool.tile([128, n_hc * tok], bf16, tag="hT")

            # ---- DMA issue order: w1, b2, x tiles, w2 chunks ----
            nc.sync.dma_start(w1_sb[:, :], w1d[:, :])
            nc.sync.dma_start(b2_sb[:, :], b2d[:, :])
            for dt in range(n_dt):
                nc.sync.dma_start(
                    xT_sb[:, dt * xstride:(dt + 1) * xstride], xT[dt])
            for c in range(8):  # 2 dt per chunk
                nc.sync.dma_start(
                    w2_sb[:, c * 2048:(c + 1) * 2048],
                    w2d[:, c * 2048:(c + 1) * 2048])

            def x_slice(dt, col, n):
                return xT_sb[:, dt * xstride + col: dt * xstride + col + n]

            # ---- mm1 (dt-outer): hT = silu(w1.T @ xT) ----
            ps1 = [ppool.tile([128, P], f32, tag="ps", name=f"ps1_{i}")
                   for i in range(4)]
            for dt in range(n_dt):
                for hc in range(n_hc):
                    for tcp in range(2):
                        for half in range(2):
                            tci = tcp * 2 + half
                            nc.tensor.matmul(
                                ps1[hc * 2 + tcp][:, half * 512:(half + 1) * 512],
                                w1_sb[:, dt * h + hc * 128: dt * h + hc * 128 + 128],
                                x_slice(dt, HALO + tci * 512, 512),
                                start=(dt == 0), stop=(dt == n_dt - 1),
                            )
            for hc in range(n_hc):
                for tcp in range(2):
                    nc.scalar.activation(
                        hT_sb[:, hc * tok + tcp * P: hc * tok + (tcp + 1) * P],
                        ps1[hc * 2 + tcp][:], AF.Silu)

            # ---- mm2 + conv + silu, 4-deep software pipeline over groups ----
            # per-group state kept across pipeline stages
            st = [None] * NG  # dict per group

            def bias(dt, w):
                return b2_sb[:, dt * W + w: dt * W + w + 1]

            for g in range(NG + 3):
                # ---- stage A (group g): matmuls + tap0 stt + tap1-3 evac ----
                if g < NG:
                    dt, pi = divmod(g, n_pi)
                    j0 = pi * P
                    kws = [ppool.tile([128, P], f32, tag="ps",
                                      name=f"kw_{g}_{w}") for w in range(W)]
                    for w in range(W):
                        for hc in range(n_hc):
                            for tcj in range(2):
                                nc.tensor.matmul(
                                    kws[w][:, tcj * 512:(tcj + 1) * 512],
                                    w2_sb[:, dt * 1024 + hc * 512 + w * 128:
                                          dt * 1024 + hc * 512 + w * 128 + 128],
                                    hT_sb[:, hc * tok + j0 + tcj * 512:
                                          hc * tok + j0 + (tcj + 1) * 512],
                                    start=(hc == 0), stop=(hc == n_hc - 1),
                                )
                    m = wpool.tile([128, 4 * P], bf16, tag="m")
                    # DVE: tap0 fused (k0 + b0) * x0  (psum read, 1x)
                    nc.vector.scalar_tensor_tensor(
                        m[:, 0:P], kws[0][:], bias(dt, 0),
                        x_slice(dt, j0 + 0, P), op0=ALU.add, op1=ALU.mult)
                    # ACT: evac taps 1-3 with bias -> bf16 SBUF
                    kb = wpool.tile([128, 3 * P], bf16, tag="kb")
                    for w in range(1, W):
                        nc.scalar.add(kb[:, (w - 1) * P: w * P], kws[w][:],
                                      bias(dt, w))
                    # GPS: odd-shift copy of x (keeps products 4B-aligned)
                    xs = wpool.tile([128, P + 4], bf16, tag="xs")
                    nc.gpsimd.tensor_copy(xs[:, 0:P + 2],
                                          x_slice(dt, j0 + 1, P + 2))
                    st[g] = dict(dt=dt, j0=j0, m=m, kb=kb, xs=xs)

                # ---- stage B (group g-1): products + first add ----
                if 0 <= g - 1 < NG:
                    s = st[g - 1]
                    dt1, j1 = s["dt"], s["j0"]
                    m, kb, xs = s["m"], s["kb"], s["xs"]
                    # DVE products (bf16 sbuf, 2x): x offsets made even via xs
                    nc.vector.tensor_mul(m[:, P:2 * P], kb[:, 0:P],
                                         xs[:, 0:P])                  # w=1
                    nc.vector.tensor_mul(m[:, 2 * P:3 * P], kb[:, P:2 * P],
                                         x_slice(dt1, j1 + 2, P))     # w=2
                    nc.vector.tensor_mul(m[:, 3 * P:4 * P], kb[:, 2 * P:3 * P],
                                         xs[:, 2:P + 2])              # w=3
                    a01 = wpool.tile([128, P], bf16, tag="a01")
                    nc.vector.tensor_add(a01[:], m[:, 0:P], m[:, P:2 * P])
                    s["a01"] = a01

                # ---- stage C (group g-2): m2+m3 (GPS), final add (DVE) ----
                if 0 <= g - 2 < NG:
                    s = st[g - 2]
                    m = s["m"]
                    a23 = wpool.tile([128, P], bf16, tag="a23")
                    nc.gpsimd.tensor_add(a23[:], m[:, 2 * P:3 * P],
                                         m[:, 3 * P:4 * P])
                    acc = wpool.tile([128, P], bf16, tag="acc")
                    nc.vector.tensor_add(acc[:], s["a01"][:], a23[:])
                    s["acc"] = acc

                # ---- stage D (group g-3): silu + DMA out ----
                if 0 <= g - 3 < NG:
                    s = st[g - 3]
                    dt3, pi3 = divmod(g - 3, n_pi)
                    ot = wpool.tile([128, P], bf16, tag="ot")
                    nc.scalar.activation(ot[:], s["acc"][:], AF.Silu)
                    nc.sync.dma_start(
                        outT[dt3, :, pi3 * P:(pi3 + 1) * P], ot[:])
                    st[g - 3] = None
    nc.compile()
    return nc


def _prep_shards(x, w1, w2, b2, tok, d, h, halo, xstride):
    """Host-side shard prep. Returns list of per-core in_maps."""
    import ml_dtypes
    bf16 = ml_dtypes.bfloat16

    n_dt = d // 128
    b, t, _ = x.shape
    shards_per_batch = (b * t // tok) // b

    # w1d[p, dt*h + j] = w1[dt*128+p, j]
    w1_r = np.ascontiguousarray(
        w1.reshape(n_dt, 128, h).transpose(1, 0, 2).reshape(128, n_dt * h)
    ).astype(bf16)
    # w2d[p, dt*1024 + hc*512 + w*128 + c] = w2[hc*128+p, (dt*128+c)*W + w]
    w2_4d = w2.reshape(2, 128, d, W)              # [hc, p, dcol, w]
    w2_5d = w2_4d.reshape(2, 128, n_dt, 128, W)   # [hc, p, dt, c, w]
    w2_r = np.ascontiguousarray(
        w2_5d.transpose(1, 2, 0, 4, 3)            # [p, dt, hc, w, c]
        .reshape(128, n_dt * 1024)).astype(bf16)
    # b2d[p, dt*W + w] = b2[(dt*128+p)*W + w]
    b2_r = np.ascontiguousarray(
        b2.reshape(n_dt, 128, W).transpose(1, 0, 2).reshape(128, n_dt * W)
    ).astype(np.float32)

    in_maps = []
    for core in range(N_CORES):
        bi, half = divmod(core, shards_per_batch)
        t0 = half * tok
        xh = np.zeros((tok + halo, d), np.float32)
        lo = max(t0 - halo, 0)
        xh[halo - (t0 - lo):] = x[bi, lo: t0 + tok]
        xTc = np.zeros((n_dt, 128, xstride), bf16)
        xTc[:, :, : tok + halo] = (
            xh.T.astype(bf16).reshape(n_dt, 128, tok + halo))
        in_maps.append({
            "xT": xTc, "w1d": w1_r, "w2d": w2_r, "b2d": b2_r})
    return in_maps


_NC_CACHE = {}


def kernel(x, w1, w2, b2, trace=False):
    from concourse.bass_utils import run_bass_kernel_spmd

    tok, d, h = TOK, D, H
    xstride = tok + HALO + 1  # even -> keeps bf16 4B alignment per dtile
    key = (tok, d, h)
    if key not in _NC_CACHE:
        _NC_CACHE[key] = _build_nc(tok, d, h, xstride)
    nc = _NC_CACHE[key]

    in_maps = _prep_shards(
        np.asarray(x, np.float32), np.asarray(w1, np.float32),
        np.asarray(w2, np.float32), np.asarray(b2, np.float32),
        tok, d, h, HALO, xstride)

    res = run_bass_kernel_spmd(nc, in_maps, core_ids=list(range(N_CORES)),
                               trace=trace)
    kernel.last_result = res

    shards_per_batch = (B * T // tok) // B
    out = np.empty((B, T, D), np.float32)
    for core in range(N_CORES):
        bi, half = divmod(core, shards_per_batch)
        oT = res.results[core]["outT"]  # [n_dt, 128, tok]
        out[bi, half * tok:(half + 1) * tok] = (
            oT.reshape(d, tok).T.astype(np.float32))
    return out
